# revision 18
# baseline (speedup 1.0000x reference)
"""Trainium2 Bass kernel for nn_ChamferLoss (reflection-symmetry chamfer loss).

Sharding: pure data parallel - batch b -> core b (B=8, 8 cores). Each core
computes its batch's loss; the host sums the 8 scalar partials.

Key algebraic identity: the reflection R_h is an isometric involution, so the
distance matrix d[i,j] = |x_i - R_h x_j|^2 is SYMMETRIC. Hence
sum_i min_j d + sum_j min_i d = 2 * sum_i min_j d - only ONE direction is
computed (48M distances instead of 96M).

Distance matmul: d[i,j] = sx_i + sy_j + u_i.y_j (u = -2x) on the PE with fp32
operands decomposed into 3 bf16 levels (6 kept cross products) stacked along
K=24 -> full-speed bf16 matmul at ~1e-6 accuracy. Aug tiles are built in
[128, NT, 32] point-major layout and transposed to the [32, 4096] matmul
orientation: X-side and head-0 via PE transposes + evac while PSUM/PE are
idle at startup; heads 1/2 via the DMA xbar transpose + de-stack DMAs
(zero compute-engine cost), overlapped with head-0's main loop.

Min-reduction: every block drains its 4 PSUM stripes identically - ACT
evacuates 2 stripes to SBUF, DVE min-fuses the other 2 against them
(TT(psum_fp32, sbuf) consumes two fresh stripes per op) - so the pipeline
is uniform; blocks differ only in post-SBUF work, balanced by a small LP:
- D-blocks (84): fp32 evac; the GPSIMD/Pool engine computes the rowmin with
  a free-axis MIN_INT tensor_reduce on the fp32 bits (IEEE order == int
  order for non-negative values; emitted as a raw InstTensorReduce - the
  cayman ISA allows it though the bass helper only exposes partition-axis
  reduces on Pool; float and int32 TENSOR_TENSOR min are NOT legal on Pool).
- A2-blocks (12): fp16 evac; DVE runs a 2x-mode fp16 min tree; 128-wide
  partials are batched 8 blocks per DVE tensor_reduce.
Cost-model timeline: 278 us/core, DVE/Pool ~92% busy (baseline: 713 us).
"""

import sys

sys.path.insert(0, "/opt/trn_rl_repo")

from contextlib import ExitStack

import numpy as np

import concourse.bass as bass
import concourse.bacc as bacc
import concourse.tile as tile
from concourse import mybir
from concourse.masks import make_identity
from concourse.bass_utils import run_bass_kernel_spmd

F32 = mybir.dt.float32
BF16 = mybir.dt.bfloat16
FP16 = mybir.dt.float16
I32 = mybir.dt.int32
AX = mybir.AxisListType
OP = mybir.AluOpType
AF = mybir.ActivationFunctionType

P = 128
H = 3
REG_COEF = 25.0
B = 8

# level patterns for the 6 kept cross products (x-level, y-level):
# (h,h) (h,m) (h,l) (m,h) (m,m) (l,h)
L_LEVELS = [0, 0, 0, 1, 1, 2]  # x-side level per 3-row group
R_LEVELS = [0, 1, 2, 0, 1, 0]  # y-side level per 3-row group

# per-block reduction recipe mix (96 blocks total), from the engine-balance LP
N_A2 = 12
N_B2 = 0


def _recipe_pattern():
    # uniform drain: every block does 2 ACT evacs + 2 DVE fused TTs; blocks
    # differ only in post-SBUF work (A2: DVE fp16 tree; D: Pool int-reduce).
    pat = ["D"] * 96
    for j in range(N_A2):
        pat[int((j + 0.5) / N_A2 * 96)] = "A2"
    placed = 0
    i = 3
    while placed < N_B2:
        if pat[i] == "D":
            pat[i] = "B2"
            placed += 1
        i += 37
    return pat


def _split3(nc, pool, src, shape, tag):
    """3-level bf16 split of an f32 tile: src ~= b0+b1+b2 (rel ~2^-25)."""
    outs = []
    cur = src
    for lv in range(3):
        b = pool.tile(shape, BF16, tag=f"{tag}b{lv}")
        nc.scalar.copy(out=b, in_=cur)
        outs.append(b)
        if lv < 2:
            r = pool.tile(shape, F32, tag=f"{tag}r{lv}")
            nc.vector.tensor_tensor(out=r, in0=cur, in1=b, op=OP.subtract)
            cur = r
    return outs


def _pool_reduce_min_i32(nc, out_col_f32, in_f32):
    """Free-axis MIN_INT tensor_reduce on the Pool engine over fp32 bits."""
    g = nc.gpsimd
    ini = in_f32.bitcast(I32)
    outi = out_col_f32.bitcast(I32)
    return g.add_instruction(mybir.InstTensorReduce(
        name=f"I-{g.bass.next_id()}",
        op=OP.min, axis=AX.X,
        ins=[g.lower_ap(ini.opt(keep_dims=frozenset({0, len(ini.shape) - 1})),
                        opt=False)],
        outs=[g.lower_ap(outi)],
        apply_absolute_value=None, apply_transpose=None, negate=None))


def emit_chamfer(nc, n=4096):
    NT = n // P           # 32 point chunks of 128
    NQ = NT // 4          # 4-chunk transpose groups

    pts = nc.dram_tensor("pts", [n, 3], F32, kind="ExternalInput").ap()
    yp = nc.dram_tensor("yp", [H, 4], F32, kind="ExternalInput").ap()
    out = nc.dram_tensor("out", [1, 1], F32, kind="ExternalOutput").ap()

    with ExitStack() as ctx:
        tc = ctx.enter_context(tile.TileContext(nc))
        const = ctx.enter_context(tc.tile_pool(name="const", bufs=1))
        work = ctx.enter_context(tc.tile_pool(name="work", bufs=2))
        headp = ctx.enter_context(tc.tile_pool(name="headp", bufs=2))
        sb = ctx.enter_context(tc.tile_pool(name="sb", bufs=3))
        pstripe = ctx.enter_context(tc.tile_pool(
            name="pstripe", bufs=4, space="PSUM"))

        # ---- load points: Xn[p, t, c] = pts[t*128+p, c]
        Xn = const.tile([P, NT, 3], F32)
        nc.sync.dma_start(out=Xn, in_=pts.rearrange("(t p) c -> p t c", p=P))

        # ---- yp broadcast to all partitions
        ypb = const.tile([P, H, 4], F32)
        yp_b = bass.AP(tensor=yp.tensor, offset=yp.offset,
                       ap=[[0, P], [4, H], [1, 4]])
        nc.sync.dma_start(out=ypb, in_=yp_b)

        # ---- sx = |x|^2 per point
        Xsq = work.tile([P, NT, 3], F32)
        nc.vector.tensor_tensor(out=Xsq, in0=Xn, in1=Xn, op=OP.mult)
        sx = const.tile([P, NT], F32)
        nc.vector.tensor_tensor(out=sx, in0=Xsq[:, :, 0], in1=Xsq[:, :, 1],
                                op=OP.add)
        nc.vector.tensor_tensor(out=sx, in0=sx, in1=Xsq[:, :, 2], op=OP.add)

        # ---- u = -2x splits and sx splits
        U = work.tile([P, NT, 3], F32)
        nc.vector.tensor_scalar(out=U, in0=Xn, scalar1=-2.0, scalar2=None,
                                op0=OP.mult)
        ub = _split3(nc, work, U, [P, NT, 3], "u")
        sxb = _split3(nc, work, sx, [P, NT], "sx")

        # ---- X aug [P, NT, 32]: rows 0-17 u levels, 18-20 sx splits,
        # 21-23 ones, 24-31 zero pad; xbar-transpose to 4-chunk-stacked
        # XT4 [128, NQ*128] (chunk t rows at partitions (t%4)*32..+23)
        XSa = const.tile([P, NT, 32], BF16)
        nc.gpsimd.memset(XSa[:, :, 21:24], 1.0)
        nc.gpsimd.memset(XSa[:, :, 24:32], 0.0)
        for g, lv in enumerate(L_LEVELS):
            nc.gpsimd.tensor_copy(out=XSa[:, :, 3 * g:3 * g + 3], in_=ub[lv])
        for l in range(3):
            nc.gpsimd.tensor_copy(out=XSa[:, :, 18 + l], in_=sxb[l])
        id128 = const.tile([P, P], BF16)
        make_identity(nc, id128)

        def pe_transpose_side(aug, dest, groups=None):
            # startup path: PE transposes via PSUM + ACT evac (PE/PSUM idle
            # here; avoids the serialized DMA-engine chain at kernel start)
            for qq in (groups if groups is not None else range(NT // 8)):
                pt = pstripe.tile([32, 8 * P], BF16, tag="stripe", name="pt")
                for j in range(8):
                    t = qq * 8 + j
                    nc.tensor.transpose(pt[:, j * P:(j + 1) * P],
                                        aug[:, t, :], id128)
                dslc = dest[:, qq * 8 * P:(qq + 1) * 8 * P]
                if qq % 2 == 0:
                    nc.scalar.copy(out=dslc, in_=pt)
                else:
                    nc.vector.tensor_copy(out=dslc, in_=pt)

        xt = const.tile([32, n], BF16)
        pe_transpose_side(XSa, xt)

        # ---- per-head Y sides
        nhat = const.tile([P, H, 3], F32)
        YT = {}

        def emit_head_setup(h):
            # normalize head normal (exact DVE ops + ACT sqrt + Newton)
            sqn = headp.tile([P, 3], F32, tag="sqn")
            nc.vector.tensor_tensor(out=sqn, in0=ypb[:, h, 0:3],
                                    in1=ypb[:, h, 0:3], op=OP.mult)
            nn = headp.tile([P, 1], F32, tag="nn")
            nc.vector.tensor_reduce(out=nn, in_=sqn, axis=AX.X, op=OP.add)
            sq_ = headp.tile([P, 1], F32, tag="sq_")
            nc.scalar.activation(out=sq_, in_=nn, func=AF.Sqrt)
            rs0 = headp.tile([P, 1], F32, tag="rs0")
            nc.vector.reciprocal(out=rs0, in_=sq_)
            a = headp.tile([P, 1], F32, tag="nta")
            nc.vector.tensor_tensor(out=a, in0=rs0, in1=rs0, op=OP.mult)
            nc.vector.tensor_tensor(out=a, in0=a, in1=nn, op=OP.mult)
            nc.vector.tensor_scalar(out=a, in0=a, scalar1=-0.5, scalar2=1.5,
                                    op0=OP.mult, op1=OP.add)
            rs = headp.tile([P, 1], F32, tag="rs")
            nc.vector.tensor_tensor(out=rs, in0=rs0, in1=a, op=OP.mult)
            nc.vector.tensor_scalar(out=nhat[:, h, :], in0=ypb[:, h, 0:3],
                                    scalar1=rs, scalar2=None, op0=OP.mult)
            off = ypb[:, h, 3:4]

            # s[p,t] = nhat . x + off
            s = headp.tile([P, NT], F32, tag="s")
            t0 = headp.tile([P, NT], F32, tag="t0")
            nc.vector.tensor_scalar(out=s, in0=Xn[:, :, 0],
                                    scalar1=nhat[:, h, 0:1], scalar2=off,
                                    op0=OP.mult, op1=OP.add)
            nc.vector.tensor_scalar(out=t0, in0=Xn[:, :, 1],
                                    scalar1=nhat[:, h, 1:2], scalar2=None,
                                    op0=OP.mult)
            nc.vector.tensor_tensor(out=s, in0=s, in1=t0, op=OP.add)
            nc.vector.tensor_scalar(out=t0, in0=Xn[:, :, 2],
                                    scalar1=nhat[:, h, 2:3], scalar2=None,
                                    op0=OP.mult)
            nc.vector.tensor_tensor(out=s, in0=s, in1=t0, op=OP.add)

            # reflected points Yn = x - 2 s nhat ; sy = sx + 4*off*s
            m2 = headp.tile([P, 3], F32, tag="m2")
            nc.vector.tensor_scalar(out=m2, in0=nhat[:, h, :], scalar1=-2.0,
                                    scalar2=None, op0=OP.mult)
            Yn = headp.tile([P, NT, 3], F32, tag="Yn")
            tc_ = headp.tile([P, NT], F32, tag="tc_")
            for c in range(3):
                nc.vector.tensor_scalar(out=tc_, in0=s, scalar1=m2[:, c:c + 1],
                                        scalar2=None, op0=OP.mult)
                nc.vector.tensor_tensor(out=Yn[:, :, c], in0=Xn[:, :, c],
                                        in1=tc_, op=OP.add)
            o4 = headp.tile([P, 1], F32, tag="o4")
            nc.vector.tensor_scalar(out=o4, in0=off, scalar1=4.0, scalar2=None,
                                    op0=OP.mult)
            sy = headp.tile([P, NT], F32, tag="sy")
            nc.vector.tensor_scalar(out=sy, in0=s, scalar1=o4, scalar2=None,
                                    op0=OP.mult)
            nc.vector.tensor_tensor(out=sy, in0=sy, in1=sx, op=OP.add)

            # y / sy splits and Y aug [P, NT, 32]:
            # rows 0-17 y levels R, 18-20 ones, 21-23 sy splits, 24-31 pad
            yb = _split3(nc, headp, Yn, [P, NT, 3], "y")
            syb = _split3(nc, headp, sy, [P, NT], "sy")
            YSa = headp.tile([P, NT, 32], BF16, tag="YSa")
            nc.gpsimd.memset(YSa[:, :, 18:21], 1.0)
            nc.gpsimd.memset(YSa[:, :, 24:32], 0.0)
            for g, lv in enumerate(R_LEVELS):
                nc.gpsimd.tensor_copy(out=YSa[:, :, 3 * g:3 * g + 3],
                                      in_=yb[lv])
            for l in range(3):
                nc.gpsimd.tensor_copy(out=YSa[:, :, 21 + l], in_=syb[l])

            # transpose to rhs layout [32, n]: PE route for head 0 (fast
            # startup), xbar DMA route for heads 1/2 (off compute engines)
            yt = const.tile([32, n], BF16, tag=f"yt{h}")
            if h == 0:
                pe_transpose_side(YSa, yt)
            else:
                YT4 = headp.tile([P, NQ * P], BF16, tag="YT4")
                for q in range(NQ):
                    nc.sync.dma_start_transpose(
                        out=YT4[:, q * P:(q + 1) * P],
                        in_=YSa[:, 4 * q:4 * q + 4, :])
                ytv = yt.rearrange("r (q k p) -> r q k p", k=4, p=P)
                y4v = YT4.rearrange("r (q p) -> r q p", p=P)
                for k in range(4):
                    nc.sync.dma_start(out=ytv[:, :, k, :],
                                      in_=y4v[32 * k:32 * k + 32, :, :])
            YT[h] = yt

        def emit_reg(reg):
            # regularizer: needs all heads' nhat (exact DVE ops)
            gsq = work.tile([P, 9], F32, tag="gsq")
            gtmp = work.tile([P, 3], F32, tag="gtmp")
            for m in range(3):
                for nn_ in range(3):
                    nc.vector.tensor_tensor(out=gtmp, in0=nhat[:, m, :],
                                            in1=nhat[:, nn_, :], op=OP.mult)
                    g1 = gsq[:, 3 * m + nn_:3 * m + nn_ + 1]
                    nc.vector.tensor_reduce(out=g1, in_=gtmp, axis=AX.X,
                                            op=OP.add)
                    if m == nn_:
                        nc.vector.tensor_scalar(out=g1, in0=g1, scalar1=-1.0,
                                                scalar2=None, op0=OP.add)
            nc.vector.tensor_tensor(out=gsq, in0=gsq, in1=gsq, op=OP.mult)
            q = work.tile([P, 1], F32, tag="q")
            nc.vector.tensor_reduce(out=q, in_=gsq, axis=AX.X, op=OP.add)
            sq0 = work.tile([P, 1], F32, tag="sq0")
            nc.scalar.activation(out=sq0, in_=q, func=AF.Sqrt)
            rcp = work.tile([P, 1], F32, tag="rcp")
            nc.vector.reciprocal(out=rcp, in_=sq0)
            nc.vector.tensor_tensor(out=rcp, in0=rcp, in1=q, op=OP.mult)
            nc.vector.tensor_tensor(out=rcp, in0=rcp, in1=sq0, op=OP.add)
            nc.vector.tensor_scalar(out=reg, in0=rcp, scalar1=0.5 * REG_COEF,
                                    scalar2=None, op0=OP.mult)

        # ---- main loop: 96 (head, row-block) blocks, one direction only.
        # Head h+1's setup (and the regularizer) are emitted a few blocks
        # into head h's stream so engine FIFOs overlap setup with reduction.
        mins_all = const.tile([P, 2 * NT * H], F32)  # cols 96.. unused
        pattern = _recipe_pattern()
        state = {"a2_idx": 0, "other_col": N_A2, "bt": None,
                 "bt_fill": 0, "bt_base": 0}
        reg = work.tile([P, 1], F32, tag="reg")
        emit_head_setup(0)

        def emit_block(h, i, recipe, st):
                lhsT = xt[0:24, i * P:(i + 1) * P]
                ss = []
                for g in range(4):
                    ps = pstripe.tile([P, 1024], F32, tag="stripe")
                    for m in range(2):
                        nc.tensor.matmul(
                            ps[:, m * 512:(m + 1) * 512],
                            lhsT=lhsT,
                            rhs=YT[h][0:24, g * 1024 + m * 512:
                                      g * 1024 + (m + 1) * 512],
                            start=True, stop=True)
                    ss.append(ps)

                if recipe == "D":
                    e32 = sb.tile([P, 2048], F32, tag="e32", bufs=4)
                    nc.scalar.copy(out=e32[:, 0:1024], in_=ss[0])
                    nc.scalar.copy(out=e32[:, 1024:2048], in_=ss[1])
                    m32 = sb.tile([P, 2048], F32, tag="m32", bufs=6)
                    nc.vector.tensor_tensor(out=m32[:, 0:1024], in0=ss[2],
                                            in1=e32[:, 0:1024], op=OP.min)
                    nc.vector.tensor_tensor(out=m32[:, 1024:2048], in0=ss[3],
                                            in1=e32[:, 1024:2048], op=OP.min)
                    _pool_reduce_min_i32(
                        nc, mins_all[:, st["other_col"]:st["other_col"] + 1],
                        m32)
                    st["other_col"] += 1
                elif recipe == "B2":
                    e4 = sb.tile([P, 4096], F32, tag="e4k", bufs=1)
                    for g in range(4):
                        nc.scalar.copy(out=e4[:, g * 1024:(g + 1) * 1024],
                                       in_=ss[g])
                    _pool_reduce_min_i32(
                        nc, mins_all[:, st["other_col"]:st["other_col"] + 1],
                        e4)
                    st["other_col"] += 1
                else:  # A2: same drain shape as D, fp16 + DVE tree tail
                    e16 = sb.tile([P, 2048], FP16, tag="e16")
                    nc.scalar.copy(out=e16[:, 0:1024], in_=ss[0])
                    nc.scalar.copy(out=e16[:, 1024:2048], in_=ss[1])
                    m16 = sb.tile([P, 2048], FP16, tag="m16")
                    nc.vector.tensor_tensor(out=m16[:, 0:1024], in0=ss[2],
                                            in1=e16[:, 0:1024], op=OP.min)
                    nc.vector.tensor_tensor(out=m16[:, 1024:2048], in0=ss[3],
                                            in1=e16[:, 1024:2048], op=OP.min)
                    c1 = sb.tile([P, 1024], FP16, tag="c1")
                    nc.vector.tensor_tensor(out=c1, in0=m16[:, 0:1024],
                                            in1=m16[:, 1024:2048], op=OP.min)
                    c2 = sb.tile([P, 512], FP16, tag="c2")
                    nc.vector.tensor_tensor(out=c2, in0=c1[:, 0:512],
                                            in1=c1[:, 512:1024], op=OP.min)
                    c3 = sb.tile([P, 256], FP16, tag="c3")
                    nc.vector.tensor_tensor(out=c3, in0=c2[:, 0:256],
                                            in1=c2[:, 256:512], op=OP.min)
                    if st["bt"] is None:
                        st["bt"] = sb.tile([P, 8, P], F32, tag="bt", bufs=2, name="bt")
                        st["bt_fill"] = 0
                        st["bt_base"] = st["a2_idx"]
                    nc.vector.tensor_tensor(out=st["bt"][:, st["bt_fill"], :],
                                            in0=c3[:, 0:128],
                                            in1=c3[:, 128:256], op=OP.min)
                    st["bt_fill"] += 1
                    st["a2_idx"] += 1
                    if st["bt_fill"] == 8 or st["a2_idx"] == N_A2:
                        nc.vector.tensor_reduce(
                            out=mins_all[:, st["bt_base"]:
                                         st["bt_base"] + st["bt_fill"]],
                            in_=st["bt"][:, 0:st["bt_fill"], :],
                            axis=AX.X, op=OP.min)
                        st["bt"] = None

        bidx = 0
        for h in range(H):
            for i in range(NT):
                emit_block(h, i, pattern[bidx], state)
                bidx += 1
                if h < H - 1 and i == 5:
                    emit_head_setup(h + 1)
                if h == H - 1 and i == 0:
                    emit_reg(reg)

        # ---- final: 2 * sum(rowmins) + reg
        sv = work.tile([P, 1], F32, tag="sv")
        nc.vector.tensor_reduce(out=sv, in_=mins_all[:, 0:96], axis=AX.X,
                                op=OP.add)
        row = work.tile([1, P], F32, tag="foldrow")
        nc.sync.dma_start(out=row, in_=sv)
        tot = work.tile([1, 1], F32, tag="tot")
        nc.vector.tensor_reduce(out=tot, in_=row, axis=AX.X, op=OP.add)
        final = work.tile([1, 1], F32, tag="final")
        nc.vector.tensor_scalar(out=final, in0=tot, scalar1=2.0, scalar2=None,
                                op0=OP.mult)
        nc.vector.tensor_tensor(out=final, in0=final, in1=reg[0:1, :],
                                op=OP.add)
        nc.sync.dma_start(out=out, in_=final)


_CACHE = {}


def _get_nc(n=4096):
    if n not in _CACHE:
        nc = bacc.Bacc("TRN2", target_bir_lowering=False, debug=False,
                       num_devices=B)
        emit_chamfer(nc, n)
        nc.compile()
        _CACHE[n] = nc
    return _CACHE[n]


def kernel(sample_points: np.ndarray, y_pred: np.ndarray) -> np.ndarray:
    assert sample_points.shape == (B, 4096, 3)
    assert y_pred.shape == (B, H, 4)
    nc = _get_nc(4096)
    in_maps = [
        {"pts": np.ascontiguousarray(sample_points[b], dtype=np.float32),
         "yp": np.ascontiguousarray(y_pred[b], dtype=np.float32)}
        for b in range(B)
    ]
    # the axon-tunneled device pool occasionally reports a transiently
    # wedged core; retry a few times before giving up
    import time as _time
    last_err = None
    for attempt in range(4):
        try:
            res = run_bass_kernel_spmd(nc, in_maps, list(range(B)))
            break
        except Exception as e:  # noqa: BLE001
            last_err = e
            _time.sleep(3.0 * (attempt + 1))
    else:
        raise last_err
    total = np.float64(0.0)
    for b in range(B):
        total += np.float64(res.results[b]["out"][0, 0])
    return np.asarray(total, dtype=np.float32).reshape(())
